# revision 8
# baseline (speedup 1.0000x reference)
"""AttnAggregator2 Trainium2 kernel (v2).

Math (per node n, with X[n, s, :] = table rows of [self, neigh_0..neigh_24]):
    Q       = table[node] @ Wq^T + bq
    scores  = Q . K  where K = X @ Wk^T + bk
            = (Q @ Wk) . X + (Q . bk)          <- Q.bk is constant per node and
                                                  cancels in softmax: dropped.
    attn    = softmax(scores)
    mix     = attn-weighted sum of V = (sum_s attn_s X_s) @ Wv^T + bv
                                                  (sum attn = 1 absorbs bv)

Folding further: q~ = Q @ Wk = Xself @ (Wq^T @ Wk) + (bq @ Wk) = Xself @ W' + b'
with W', b' precomputed on host. So per 128-node tile:
    gather   G[p, s, :]  = table16[idx[p, s]]   (ONE multi-offset indirect DMA
                                                 per 2 tiles; fp16 table halves
                                                 HBM traffic vs f32)
    Xself^T  via PE transpose of G[:, 0, :]
    q~       = PE matmul lhsT=Xself^T rhs=W'  (+ b' via DVE add)
    scores   = reduce_d(G * broadcast_s(q~))   (DVE fp16)
    attn     = softmax over s                  (DVE + ACT)
    diag_s   = diag(attn[:, s])                (DVE: fp16 identity x attn)
    Xmix^T   = sum_s (G_s)^T @ diag_s          (PE fp16, PSUM accum)
    out^T    = Wv @ Xmix^T + bv                (PE fp16 + ACT bias)
Output written transposed [128, n]; host transposes back.

Sharding: data-parallel over nodes, 8 cores, table + weights replicated.
"""

import sys
from contextlib import ExitStack

import numpy as np

sys.path.insert(0, "/opt/trn_rl_repo")

import concourse.bass as bass
import concourse.mybir as mybir
import concourse.tile as tile
from concourse import bacc
from concourse.bass_utils import run_bass_kernel_spmd
from concourse.masks import make_identity

F32 = mybir.dt.float32
F16 = mybir.dt.float16
I32 = mybir.dt.int32

VOCAB = 100000
N_NODES = 50000
S = 25
S1 = S + 1  # self + sampled neighbors
D = 128
P = 128
N_CORES = 8
N_PER_CORE = N_NODES // N_CORES  # 6250
N_TILES = 50  # even so gathers pair up; 50*128 = 6400 >= 6250
N_PAD = N_TILES * P  # 6400
TILES_PER_CALL = 2  # tiles gathered per indirect DMA call
S2 = TILES_PER_CALL * S1
# "multi": one multi-offset indirect DMA per 2 tiles (fast, needs working
# multi-offset ucode); "per_s": one 128-row indirect DMA per slot (slow, known
# good).
GATHER_MODE = "per_s"


def build_kernel(n_tiles: int = N_TILES, vocab: int = VOCAB):
    nc = bacc.Bacc(
        "TRN2",
        target_bir_lowering=False,
        debug=False,
        enable_asserts=False,
    )

    table = nc.dram_tensor("table", [vocab, D], F16, kind="ExternalInput").ap()
    idx = nc.dram_tensor("idx", [P, n_tiles * S1], I32, kind="ExternalInput").ap()
    wprime = nc.dram_tensor("wprime", [D, D], F32, kind="ExternalInput").ap()
    bprime = nc.dram_tensor("bprime", [P, D], F32, kind="ExternalInput").ap()
    wvT = nc.dram_tensor("wvT", [D, D], F32, kind="ExternalInput").ap()
    bv = nc.dram_tensor("bv", [D, 1], F32, kind="ExternalInput").ap()
    out = nc.dram_tensor("out", [D, n_tiles * P], F32, kind="ExternalOutput").ap()

    n_pairs = n_tiles // TILES_PER_CALL

    with tile.TileContext(nc) as tc, ExitStack() as ctx:
        const = ctx.enter_context(tc.tile_pool(name="const", bufs=1))
        gpool = ctx.enter_context(tc.tile_pool(name="gpool", bufs=3))
        prodp = ctx.enter_context(tc.tile_pool(name="prodp", bufs=2))
        diagp = ctx.enter_context(tc.tile_pool(name="diagp", bufs=2))
        small = ctx.enter_context(tc.tile_pool(name="small", bufs=4))
        outp = ctx.enter_context(tc.tile_pool(name="outp", bufs=3))
        psum = ctx.enter_context(tc.tile_pool(name="psum", bufs=2, space="PSUM"))
        psum_xm = ctx.enter_context(tc.tile_pool(name="psum_xm", bufs=2, space="PSUM"))

        ident = const.tile([P, P], F32)
        make_identity(nc, ident[:])
        ident_h = const.tile([P, P], F16)
        nc.scalar.copy(ident_h[:], ident[:])
        wprime_s = const.tile([D, D], F32)
        nc.sync.dma_start(wprime_s[:], wprime)
        wprime_h = const.tile([D, D], F16)
        nc.scalar.copy(wprime_h[:], wprime_s[:])
        # b' comes pre-replicated across partitions from the host
        bprime_s = const.tile([P, D], F32)
        nc.sync.dma_start(bprime_s[:], bprime)
        wvT_s = const.tile([D, D], F32)
        nc.sync.dma_start(wvT_s[:], wvT)
        wvT_h = const.tile([D, D], F16)
        nc.scalar.copy(wvT_h[:], wvT_s[:])
        bv_s = const.tile([D, 1], F32)
        nc.sync.dma_start(bv_s[:], bv)
        idx_all = const.tile([P, n_tiles * S1], I32)
        nc.sync.dma_start(idx_all[:], idx)

        for pair in range(n_pairs):
            # Gather 2 tiles of 128 nodes x 26 slots in ONE indirect DMA:
            # g2[p, j, :] = table16[idx[p, pair*52 + j]]
            g2 = gpool.tile([P, S2, D], F16)
            if GATHER_MODE == "multi":
                nc.gpsimd.indirect_dma_start(
                    out=g2[:, :, :],
                    out_offset=None,
                    in_=table,
                    in_offset=bass.IndirectOffsetOnAxis(
                        ap=idx_all[:, pair * S2 : (pair + 1) * S2], axis=0
                    ),
                    oob_is_err=False,
                )
            else:
                for s in range(S2):
                    nc.gpsimd.indirect_dma_start(
                        out=g2[:, s, :],
                        out_offset=None,
                        in_=table,
                        in_offset=bass.IndirectOffsetOnAxis(
                            ap=idx_all[:, pair * S2 + s : pair * S2 + s + 1], axis=0
                        ),
                        oob_is_err=False,
                    )

            for half in range(TILES_PER_CALL):
                t = pair * TILES_PER_CALL + half
                g = g2[:, half * S1 : (half + 1) * S1, :]  # [P, S1, D] fp16

                # Xself^T via PE transpose (fp16 in, fp16 psum out)
                ps_xsT = psum.tile([P, P], F16)
                nc.tensor.transpose(ps_xsT[:], g[:, 0, :], ident_h[:])
                xsT = small.tile([P, P], F16)
                nc.scalar.copy(xsT[:], ps_xsT[:])

                # q~ = Xself @ W' + b'   [n, d]
                ps_q = psum.tile([P, P], F32)
                nc.tensor.matmul(
                    ps_q[:], lhsT=xsT[:], rhs=wprime_h[:], start=True, stop=True
                )
                qp = small.tile([P, P], F16)
                nc.vector.tensor_tensor(
                    qp[:], ps_q[:], bprime_s[:], op=mybir.AluOpType.add
                )

                # scores_s[n] = sum_d G[n, s, d] * q~[n, d]
                prod = prodp.tile([P, S1, D], F16)
                nc.vector.tensor_tensor(
                    prod[:],
                    g,
                    qp[:, None, :].to_broadcast([P, S1, D]),
                    op=mybir.AluOpType.mult,
                )
                sc = small.tile([P, S1], F32)
                nc.vector.tensor_reduce(
                    sc[:], prod[:], axis=mybir.AxisListType.X, op=mybir.AluOpType.add
                )

                # softmax over s (free dim)
                negmax = small.tile([P, 1], F32)
                nc.vector.tensor_reduce(
                    negmax[:],
                    sc[:],
                    axis=mybir.AxisListType.X,
                    op=mybir.AluOpType.max,
                    negate=True,
                )
                e = small.tile([P, S1], F32)
                zsum = small.tile([P, 1], F32)
                nc.scalar.activation(
                    e[:],
                    sc[:],
                    func=mybir.ActivationFunctionType.Exp,
                    bias=negmax[:, :1],
                    accum_out=zsum[:],
                )
                zinv = small.tile([P, 1], F32)
                nc.vector.reciprocal(zinv[:], zsum[:])
                attn = small.tile([P, S1], F16)
                nc.vector.tensor_scalar_mul(attn[:], e[:], zinv[:, :1])

                # diag_all[p, s, y] = attn[p, s] if p == y else 0
                diag = diagp.tile([P, S1, D], F16)
                nc.vector.tensor_tensor(
                    diag[:],
                    ident_h[:, None, :].to_broadcast([P, S1, D]),
                    attn[:, :, None].to_broadcast([P, S1, D]),
                    op=mybir.AluOpType.mult,
                )

                # Xmix^T = sum_s (G_s)^T @ diag(attn_s)   [d, n]
                ps_xm = psum_xm.tile([P, P], F32)
                for s in range(S1):
                    nc.tensor.matmul(
                        ps_xm[:],
                        lhsT=g[:, s, :],
                        rhs=diag[:, s, :],
                        start=(s == 0),
                        stop=(s == S1 - 1),
                    )
                xmT = small.tile([P, P], F16)
                nc.scalar.copy(xmT[:], ps_xm[:])

                # out^T = Wv @ Xmix^T + bv   [j, n]
                ps_mx = psum.tile([P, P], F32)
                nc.tensor.matmul(
                    ps_mx[:], lhsT=wvT_h[:], rhs=xmT[:], start=True, stop=True
                )
                o_t = outp.tile([P, P], F32)
                nc.scalar.activation(
                    o_t[:],
                    ps_mx[:],
                    func=mybir.ActivationFunctionType.Identity,
                    bias=bv_s[:, :1],
                )
                nc.sync.dma_start(out[:, bass.ts(t, P)], o_t[:])

    nc.compile()
    return nc


_NC_CACHE = {}


def _get_nc():
    key = (N_TILES, VOCAB)
    if key not in _NC_CACHE:
        _NC_CACHE[key] = build_kernel()
    return _NC_CACHE[key]


def prepare_in_maps(inputs) -> list:
    """Host-side preprocessing: fp16 table, packed int32 indices, folded
    weights. Returns one input dict per core."""
    table16 = np.ascontiguousarray(np.asarray(inputs["table"], dtype=np.float16))
    node = np.asarray(inputs["node"]).astype(np.int32)
    neighs = np.asarray(inputs["neighs"]).astype(np.int32)
    Wq = np.asarray(inputs["Wq"], dtype=np.float32)
    bq = np.asarray(inputs["bq"], dtype=np.float32)
    Wk = np.asarray(inputs["Wk"], dtype=np.float32)
    Wv = np.asarray(inputs["Wv"], dtype=np.float32)
    bv = np.asarray(inputs["bv"], dtype=np.float32)

    idx_full = np.concatenate([node[:, None], neighs], axis=1)  # [N, S1] int32

    common = {
        "table": table16,
        "wprime": np.ascontiguousarray(Wq.T @ Wk),
        "bprime": np.ascontiguousarray(np.broadcast_to((bq @ Wk)[None, :], (P, D))),
        "wvT": np.ascontiguousarray(Wv.T),
        "bv": np.ascontiguousarray(bv[:, None]),
    }

    in_maps = []
    for c in range(N_CORES):
        idx_c = idx_full[c * N_PER_CORE : (c + 1) * N_PER_CORE]
        idx_pad = np.zeros((N_PAD, S1), dtype=np.int32)
        idx_pad[:N_PER_CORE] = idx_c
        in_maps.append(dict(common, idx=np.ascontiguousarray(
            idx_pad.reshape(N_TILES, P, S1).transpose(1, 0, 2).reshape(P, N_TILES * S1)
        )))
    return in_maps


def kernel(**inputs) -> np.ndarray:
    in_maps = prepare_in_maps(inputs)
    nc = _get_nc()
    results = run_bass_kernel_spmd(nc, in_maps, list(range(N_CORES))).results

    out = np.empty((N_NODES, D), dtype=np.float32)
    for c in range(N_CORES):
        out[c * N_PER_CORE : (c + 1) * N_PER_CORE] = results[c]["out"][
            :, :N_PER_CORE
        ].T
    return out


if __name__ == "__main__":
    rng = np.random.default_rng(0)
    inputs = {
        "table": rng.standard_normal((VOCAB, D), dtype=np.float32),
        "node": rng.integers(0, VOCAB, (N_NODES,)),
        "neighs": rng.integers(0, VOCAB, (N_NODES, S)),
        "Wq": rng.uniform(-0.09, 0.09, (D, D)).astype(np.float32),
        "bq": rng.uniform(-0.09, 0.09, (D,)).astype(np.float32),
        "Wk": rng.uniform(-0.09, 0.09, (D, D)).astype(np.float32),
        "bk": rng.uniform(-0.09, 0.09, (D,)).astype(np.float32),
        "Wv": rng.uniform(-0.09, 0.09, (D, D)).astype(np.float32),
        "bv": rng.uniform(-0.09, 0.09, (D,)).astype(np.float32),
    }
    res = kernel(**inputs)
    print("kernel ran, output shape", res.shape)


# revision 10
# speedup vs baseline: 1.6955x; 1.6955x over previous
"""AttnAggregator2 Trainium2 kernel (v3: 4-queue dma_gather).

Math per node (X = table rows of [self, neigh_0..24]):
    q~      = Xself @ (Wq^T Wk) + (bq Wk)     (host-folded W', b')
    scores  = q~ . X_s   (Q.bk cancels in softmax)
    attn    = softmax(scores)
    out     = Wv (sum_s attn_s X_s) + bv

Gather strategy: the ANT dma_gather (int16 indices, vectorized Q7 descriptor
generation, queue_num parallelism over the 4 SWDGE queue pairs) instead of
per-128-row indirect DMAs. int16 only reaches 32768 rows, so the fp16 table
is addressed in 4 chunks of 32768 rows; each node's 25 neighbor slots are
bucketed by chunk (order within a node is permutation-invariant). Nodes are
profile-sorted so the 128 nodes of a tile have similar per-chunk counts;
each (tile, chunk) becomes K columns of 128 rows (short nodes padded with
in-chunk dummy rows, masked out of the softmax with an additive -30000).
Self rows keep fixed positions via one classic indirect DMA per tile
(mainline queue 0, concurrent with the gathers on queues 1-3).

The kernel structure (per-tile column counts) is data-dependent: it is
compiled per input layout and shared SPMD across the 8 cores (schedule =
max over cores). Output is written transposed [128, n]; the host transposes
and un-permutes the node sort.
"""

import sys
from contextlib import ExitStack

import numpy as np

sys.path.insert(0, "/opt/trn_rl_repo")

import concourse.bass as bass
import concourse.mybir as mybir
import concourse.tile as tile
from concourse import bacc
from concourse.bass_utils import run_bass_kernel_spmd
from concourse.masks import make_identity

F32 = mybir.dt.float32
F16 = mybir.dt.float16
I32 = mybir.dt.int32
I16 = mybir.dt.int16

VOCAB = 100000
N_NODES = 50000
S = 25
S1 = S + 1
D = 128
P = 128
N_CORES = 8
N_PER_CORE = N_NODES // N_CORES  # 6250
N_TILES = 49
N_PAD = N_TILES * P  # 6272
CHUNK = 32768
N_CHUNKS = 4
CHUNK_ROWS = [CHUNK, CHUNK, CHUNK, VOCAB - 3 * CHUNK]
MAX_CALL_COLS = 8  # 1024 indices per dma_gather call (ring capacity limit)
MASK_NEG = -30000.0


def _plan_layout(neighs_by_core):
    """Shared SPMD layout: per-core node sort orders + the shared per-tile
    per-chunk column schedule K[t][c] (max over cores)."""
    orders = []
    kcs = []
    for nb in neighs_by_core:  # [N_PER_CORE, S] int
        kc = np.stack([((nb // CHUNK) == c).sum(1) for c in range(N_CHUNKS)], 1)
        pad = N_PAD - len(nb)
        kc = np.vstack([kc, np.tile([[S, 0, 0, 0]], (pad, 1))])
        key2 = kc[:, 1].copy()
        key2[kc[:, 0] % 2 == 1] = S - key2[kc[:, 0] % 2 == 1]
        order = np.lexsort((kc[:, 3], kc[:, 2], key2, kc[:, 0]))
        orders.append(order)
        kcs.append(kc[order])
    K = np.stack([k.reshape(N_TILES, P, N_CHUNKS).max(1) for k in kcs]).max(0)
    return orders, K  # K: [N_TILES, N_CHUNKS] shared schedule


def _plan_calls(K):
    """Static call list from the schedule: per tile, per chunk, split into
    <=MAX_CALL_COLS column calls. Returns (calls, ct_per_tile, i16_total,
    col_total): calls = list of (tile, chunk, colbase_in_tile, ncols,
    i16_off)."""
    calls = []
    ct = K.sum(1)  # columns per tile
    i16_off = 0
    for t in range(N_TILES):
        colbase = 0
        for c in range(N_CHUNKS):
            k = int(K[t, c])
            j = 0
            while j < k:
                n = min(MAX_CALL_COLS, k - j)
                calls.append((t, c, colbase + j, n, i16_off))
                i16_off += n * P // 16
                j += n
            colbase += k
    return calls, ct.astype(int), i16_off, int(ct.sum())


def build_kernel(K, vocab=VOCAB):
    calls, ct, i16_total, col_total = _plan_calls(K)
    ct_max = int(ct.max())

    nc = bacc.Bacc(
        "TRN2",
        target_bir_lowering=False,
        debug=False,
        enable_asserts=False,
        num_swdge_queues=4,
    )

    table = nc.dram_tensor("table", [vocab, D], F16, kind="ExternalInput").ap()
    idx16 = nc.dram_tensor("idx16", [P, i16_total], I16, kind="ExternalInput").ap()
    selfidx = nc.dram_tensor("selfidx", [P, N_TILES], I32, kind="ExternalInput").ap()
    maskd = nc.dram_tensor("mask", [P, col_total], F32, kind="ExternalInput").ap()
    wprime = nc.dram_tensor("wprime", [D, D], F32, kind="ExternalInput").ap()
    bprime = nc.dram_tensor("bprime", [P, D], F32, kind="ExternalInput").ap()
    wvT = nc.dram_tensor("wvT", [D, D], F32, kind="ExternalInput").ap()
    bv = nc.dram_tensor("bv", [D, 1], F32, kind="ExternalInput").ap()
    out = nc.dram_tensor("out", [D, N_TILES * P], F32, kind="ExternalOutput").ap()

    # per-tile call slices
    tile_calls = [[] for _ in range(N_TILES)]
    for t, c, colbase, ncols, off in calls:
        tile_calls[t].append((c, colbase, ncols, off))

    with tile.TileContext(nc) as tc, ExitStack() as ctx:
        const = ctx.enter_context(tc.tile_pool(name="const", bufs=1))
        gpool = ctx.enter_context(tc.tile_pool(name="gpool", bufs=3))
        prodp = ctx.enter_context(tc.tile_pool(name="prodp", bufs=2))
        diagp = ctx.enter_context(tc.tile_pool(name="diagp", bufs=2))
        small = ctx.enter_context(tc.tile_pool(name="small", bufs=4))
        outp = ctx.enter_context(tc.tile_pool(name="outp", bufs=3))
        psum = ctx.enter_context(tc.tile_pool(name="psum", bufs=2, space="PSUM"))
        psum_xm = ctx.enter_context(tc.tile_pool(name="psum_xm", bufs=2, space="PSUM"))

        ident = const.tile([P, P], F32)
        make_identity(nc, ident[:])
        ident_h = const.tile([P, P], F16)
        nc.scalar.copy(ident_h[:], ident[:])
        wprime_s = const.tile([D, D], F32)
        nc.sync.dma_start(wprime_s[:], wprime)
        wprime_h = const.tile([D, D], F16)
        nc.scalar.copy(wprime_h[:], wprime_s[:])
        bprime_s = const.tile([P, D], F32)
        nc.sync.dma_start(bprime_s[:], bprime)
        wvT_s = const.tile([D, D], F32)
        nc.sync.dma_start(wvT_s[:], wvT)
        wvT_h = const.tile([D, D], F16)
        nc.scalar.copy(wvT_h[:], wvT_s[:])
        bv_s = const.tile([D, 1], F32)
        nc.sync.dma_start(bv_s[:], bv)
        idx16_s = const.tile([P, i16_total], I16)
        nc.sync.dma_start(idx16_s[:], idx16)
        selfidx_s = const.tile([P, N_TILES], I32)
        nc.sync.dma_start(selfidx_s[:], selfidx)
        mask_s = const.tile([P, col_total], F32)
        nc.sync.dma_start(mask_s[:], maskd)
        selfbuf = const.tile([P, N_TILES, D], F16)

        qrr = 0
        mask_off = 0
        for t in range(N_TILES):
            CT = int(ct[t])

            # self rows: classic indirect DMA (mainline queue 0)
            nc.gpsimd.indirect_dma_start(
                out=selfbuf[:, t, :],
                out_offset=None,
                in_=table,
                in_offset=bass.IndirectOffsetOnAxis(
                    ap=selfidx_s[:, t : t + 1], axis=0
                ),
                oob_is_err=False,
            )

            # neighbor columns: dma_gather per (tile, chunk) split to <=8 cols
            g = gpool.tile([P, ct_max, D], F16)
            for c, colbase, ncols, off in tile_calls[t]:
                nidx = ncols * P
                nc.gpsimd.dma_gather(
                    g[:, colbase : colbase + ncols, :],
                    table[c * CHUNK : c * CHUNK + CHUNK_ROWS[c], :],
                    idx16_s[:, off : off + ncols * P // 16],
                    nidx,
                    nidx,
                    D,
                    elem_step=D,
                    transpose=False,
                    queue_num=1 + (qrr % 3) if (qrr % 4) != 3 else 0,
                )
                qrr += 1

            # Xself^T via PE transpose (fp16)
            ps_xsT = psum.tile([P, P], F16)
            nc.tensor.transpose(ps_xsT[:], selfbuf[:, t, :], ident_h[:])
            xsT = small.tile([P, P], F16)
            nc.scalar.copy(xsT[:], ps_xsT[:])

            # q~ = Xself @ W' + b'
            ps_q = psum.tile([P, P], F32)
            nc.tensor.matmul(
                ps_q[:], lhsT=xsT[:], rhs=wprime_h[:], start=True, stop=True
            )
            qp = small.tile([P, P], F16)
            nc.vector.tensor_tensor(
                qp[:], ps_q[:], bprime_s[:], op=mybir.AluOpType.add
            )

            # neighbor scores
            prod = prodp.tile([P, ct_max, D], F16)
            nc.vector.tensor_tensor(
                prod[:, :CT, :],
                g[:, :CT, :],
                qp[:, None, :].to_broadcast([P, CT, D]),
                op=mybir.AluOpType.mult,
            )
            sc = small.tile([P, ct_max + 1], F32)
            nc.vector.tensor_reduce(
                sc[:, :CT],
                prod[:, :CT, :],
                axis=mybir.AxisListType.X,
                op=mybir.AluOpType.add,
            )
            # self score
            prod_s = small.tile([P, D], F16)
            nc.vector.tensor_tensor(
                prod_s[:], selfbuf[:, t, :], qp[:], op=mybir.AluOpType.mult
            )
            nc.vector.tensor_reduce(
                sc[:, CT : CT + 1],
                prod_s[:],
                axis=mybir.AxisListType.X,
                op=mybir.AluOpType.add,
            )

            # masked scores: scm = sc*1 + mask (dummies -> -30000); self col
            # copied unmasked
            scm = small.tile([P, ct_max + 1], F32)
            nc.vector.scalar_tensor_tensor(
                scm[:, :CT],
                sc[:, :CT],
                1.0,
                mask_s[:, mask_off : mask_off + CT],
                op0=mybir.AluOpType.mult,
                op1=mybir.AluOpType.add,
            )
            nc.vector.tensor_copy(scm[:, CT : CT + 1], sc[:, CT : CT + 1])
            mask_off += CT

            # softmax over CT+1
            negmax = small.tile([P, 1], F32)
            nc.vector.tensor_reduce(
                negmax[:],
                scm[:, : CT + 1],
                axis=mybir.AxisListType.X,
                op=mybir.AluOpType.max,
                negate=True,
            )
            e = small.tile([P, ct_max + 1], F32)
            zsum = small.tile([P, 1], F32)
            nc.scalar.activation(
                e[:, : CT + 1],
                scm[:, : CT + 1],
                func=mybir.ActivationFunctionType.Exp,
                bias=negmax[:, :1],
                accum_out=zsum[:],
            )
            zinv = small.tile([P, 1], F32)
            nc.vector.reciprocal(zinv[:], zsum[:])
            attn = small.tile([P, ct_max + 1], F16)
            nc.vector.tensor_scalar_mul(
                attn[:, : CT + 1], e[:, : CT + 1], zinv[:, :1]
            )

            # diag_all[p, j, y] = attn[p, j] * (p == y)
            diag = diagp.tile([P, ct_max + 1, D], F16)
            nc.vector.tensor_tensor(
                diag[:, : CT + 1, :],
                ident_h[:, None, :].to_broadcast([P, CT + 1, D]),
                attn[:, : CT + 1, None].to_broadcast([P, CT + 1, D]),
                op=mybir.AluOpType.mult,
            )

            # Xmix^T = sum_j G_j^T @ diag_j  (+ self column)
            ps_xm = psum_xm.tile([P, P], F32)
            for j in range(CT):
                nc.tensor.matmul(
                    ps_xm[:],
                    lhsT=g[:, j, :],
                    rhs=diag[:, j, :],
                    start=(j == 0),
                    stop=False,
                )
            nc.tensor.matmul(
                ps_xm[:],
                lhsT=selfbuf[:, t, :],
                rhs=diag[:, CT, :],
                start=False,
                stop=True,
            )
            xmT = small.tile([P, P], F16)
            nc.scalar.copy(xmT[:], ps_xm[:])

            # out^T = Wv @ Xmix^T + bv
            ps_mx = psum.tile([P, P], F32)
            nc.tensor.matmul(
                ps_mx[:], lhsT=wvT_h[:], rhs=xmT[:], start=True, stop=True
            )
            o_t = outp.tile([P, P], F32)
            nc.scalar.activation(
                o_t[:],
                ps_mx[:],
                func=mybir.ActivationFunctionType.Identity,
                bias=bv_s[:, :1],
            )
            nc.sync.dma_start(out[:, bass.ts(t, P)], o_t[:])

    nc.compile()
    return nc


_NC_CACHE = {}


def _get_nc_for(K):
    key = tuple(K.flatten().tolist())
    if key not in _NC_CACHE:
        _NC_CACHE[key] = build_kernel(K)
    return _NC_CACHE[key]


def _get_nc():
    """test.py hook: returns the nc compiled by the last kernel() call."""
    assert _NC_CACHE, "call kernel() first"
    return next(iter(_NC_CACHE.values()))


def prepare_in_maps(inputs):
    """Host preprocessing. Returns (in_maps, orders, K)."""
    table16 = np.ascontiguousarray(np.asarray(inputs["table"], dtype=np.float16))
    node = np.asarray(inputs["node"]).astype(np.int64)
    neighs = np.asarray(inputs["neighs"]).astype(np.int64)
    Wq = np.asarray(inputs["Wq"], dtype=np.float32)
    bq = np.asarray(inputs["bq"], dtype=np.float32)
    Wk = np.asarray(inputs["Wk"], dtype=np.float32)
    Wv = np.asarray(inputs["Wv"], dtype=np.float32)
    bv = np.asarray(inputs["bv"], dtype=np.float32)

    neighs_by_core = [
        neighs[c * N_PER_CORE : (c + 1) * N_PER_CORE] for c in range(N_CORES)
    ]
    orders, K = _plan_layout(neighs_by_core)
    calls, ct, i16_total, col_total = _plan_calls(K)

    common = {
        "table": table16,
        "wprime": np.ascontiguousarray(Wq.T @ Wk),
        "bprime": np.ascontiguousarray(np.broadcast_to((bq @ Wk)[None, :], (P, D))),
        "wvT": np.ascontiguousarray(Wv.T),
        "bv": np.ascontiguousarray(bv[:, None]),
    }

    in_maps = []
    for core in range(N_CORES):
        nb = neighs_by_core[core]
        nd = node[core * N_PER_CORE : (core + 1) * N_PER_CORE]
        pad = N_PAD - len(nd)
        nd_p = np.concatenate([nd, np.zeros(pad, dtype=nd.dtype)])
        nb_p = np.vstack([nb, np.zeros((pad, S), dtype=nb.dtype)])
        order = orders[core]
        nd_s = nd_p[order]
        nb_s = nb_p[order]

        # per node: neighbor lists bucketed by chunk, chunk-sorted
        chunk_of = nb_s // CHUNK  # [N_PAD, S]
        selfmat = nd_s.reshape(N_TILES, P).T.astype(np.int32)  # [P, N_TILES]

        idx16_all = np.zeros((P, i16_total), dtype=np.int16)
        mask = np.full((P, col_total), MASK_NEG, dtype=np.float32)

        # build per (tile, chunk) column entries
        # ent_store[(t, c)] : [P, K[t,c]] relative indices (int16)
        ent_store = {}
        mask_col_off = np.concatenate([[0], np.cumsum(ct)])
        for t in range(N_TILES):
            rows = slice(t * P, (t + 1) * P)
            nb_t = nb_s[rows]  # [P, S]
            co_t = chunk_of[rows]  # [P, S]
            colbase = 0
            for c in range(N_CHUNKS):
                k = int(K[t, c])
                if k == 0:
                    continue
                ent = np.zeros((P, k), dtype=np.int16)
                val = np.zeros((P, k), dtype=bool)
                for p in range(P):
                    sel = nb_t[p][co_t[p] == c] - c * CHUNK
                    ent[p, : len(sel)] = sel.astype(np.int16)
                    val[p, : len(sel)] = True
                gcol = mask_col_off[t] + colbase
                mask[:, gcol : gcol + k] = np.where(val, 0.0, MASK_NEG)
                ent_store[(t, c)] = (ent, colbase)
                colbase += k

        # pack int16 indices per call (16-partition wrap, replicated x8)
        for t, c, colbase, ncols, off in calls:
            ent, cb = ent_store[(t, c)]
            j0 = colbase - cb
            e = ent[:, j0 : j0 + ncols]  # [P, ncols]
            flat = e.T.ravel()  # flat[cc*128 + p]
            packed = flat.reshape(-1, 16).T  # [16, nidx/16]
            idx16_all[:, off : off + ncols * P // 16] = np.tile(packed, (8, 1))

        in_maps.append(
            dict(
                common,
                idx16=idx16_all,
                selfidx=np.ascontiguousarray(selfmat),
                mask=np.ascontiguousarray(mask),
            )
        )
    return in_maps, orders, K


def kernel(**inputs) -> np.ndarray:
    in_maps, orders, K = prepare_in_maps(inputs)
    nc = _get_nc_for(K)
    results = run_bass_kernel_spmd(nc, in_maps, list(range(N_CORES))).results

    out = np.empty((N_NODES, D), dtype=np.float32)
    for c in range(N_CORES):
        res = results[c]["out"]  # [D, N_TILES*P]
        order = orders[c]
        valid = order < N_PER_CORE
        out[c * N_PER_CORE + order[valid]] = res[:, np.where(valid)[0]].T
    return out
